# revision 16
# baseline (speedup 1.0000x reference)
"""Multi-head self-attention (B=1, S=4096, D=1024, H=16, DK=64) on 8 Trainium2
NeuronCores.

Sharding: tensor(model)-parallel over heads - 2 heads per core. Each core
computes Q^T/K^T/V^T for its 2 heads from the (host-pre-transposed) full x^T,
runs causal flash-style attention fully in transposed space (scores S^T with
keys on partitions, queries on the free dim; softmax sums come free via a
ones-column appended to V), normalizes the per-head outputs on the producer
side, then exchanges them with per-group AllToAlls (bf16 payload) so every
core ends up with all 16 heads' outputs for its own 512-query-row shard,
against which it runs the output projection.

Pipeline: one key-block per slot. Emission order per slot is
  scores(s) -> exp(s) [ScalarE] -> QKV-prefetch bursts / deferred tail work
  -> AV(s-1) -> mask(s)
so the in-order PE queue always has dependency-free work (bursts, output
projections) queued BEFORE the AV matmul that waits on the Scalar engine's
exp. The PE therefore streams continuously, which keeps the HAM activity
monitor at K=8/8 (2.4 GHz) instead of the cold 4/8 half-clock.
"""

import numpy as np
from contextlib import ExitStack

import concourse.bass as bass
import concourse.bacc as bacc
import concourse.tile as tile
import concourse.mybir as mybir
from concourse.bass_utils import run_bass_kernel_spmd
from concourse.masks import make_identity

F32 = mybir.dt.float32
BF16 = mybir.dt.bfloat16
EXP = mybir.ActivationFunctionType.Exp

N_CORES = 8
D = 1024
H = 16
DK = 64        # head dim
HPC = H // N_CORES          # heads per core (2)
QC = 512                    # query-chunk width (free dim of S^T tiles)


def build(S=4096):
    """Build + compile the SPMD program (identical on all 8 cores)."""
    SC = S // QC            # query chunks
    NSB = S // 128          # 128-wide seq blocks
    QPER = S // N_CORES     # output rows per core

    nc = bacc.Bacc("TRN2", target_bir_lowering=False, debug=False,
                   enable_asserts=False, num_devices=N_CORES)

    xt = nc.dram_tensor("xt", [D, S], BF16, kind="ExternalInput")
    wq = nc.dram_tensor("wq", [D, 128], BF16, kind="ExternalInput")
    wk = nc.dram_tensor("wk", [D, 128], BF16, kind="ExternalInput")
    wv = nc.dram_tensor("wv", [D, 128], BF16, kind="ExternalInput")
    wo = nc.dram_tensor("wo", [D, D], BF16, kind="ExternalInput")
    bq = nc.dram_tensor("bq", [128], F32, kind="ExternalInput")
    bk = nc.dram_tensor("bk", [128], F32, kind="ExternalInput")
    bv = nc.dram_tensor("bv", [128], F32, kind="ExternalInput")
    bo = nc.dram_tensor("bo", [D], BF16, kind="ExternalInput")
    out = nc.dram_tensor("out", [QPER, D], F32, kind="ExternalOutput")

    with tile.TileContext(nc) as tc, ExitStack() as ctx:
        sb = ctx.enter_context(tc.tile_pool(name="sb", bufs=1))
        sbx = ctx.enter_context(tc.tile_pool(name="sbx", bufs=2))
        sbpt = ctx.enter_context(tc.tile_pool(name="sbpt", bufs=3))
        sbtmp = ctx.enter_context(tc.tile_pool(name="sbtmp", bufs=3))
        sbnrm = ctx.enter_context(tc.tile_pool(name="sbnrm", bufs=2))
        # PSUM: one 3-slot pool of [128,1024] tiles (6 banks) shared by all
        # phases + a single [65,1024] accumulator tile (2 banks) = 8 banks.
        ps_big = ctx.enter_context(tc.tile_pool(name="ps_big", bufs=3, space="PSUM"))
        ps_ot = ctx.enter_context(tc.tile_pool(name="ps_ot", bufs=1, space="PSUM"))
        dram = ctx.enter_context(tc.tile_pool(name="dram", bufs=1, space="DRAM"))

        xt_r = xt.ap().rearrange("(t p) (c q) -> c p t q", p=128, q=QC)

        # ---- chunk-0 x^T load first (everything waits on it), split in two
        # halves so the first projection matmuls can start on t-tiles 0-3
        xt0_sb = sbx.tile([128, 8, QC], BF16, tag="xt", name="xt0")
        nc.sync.dma_start(xt0_sb[:, 0:4, :], xt_r[0][:, 0:4, :])
        nc.sync.dma_start(xt0_sb[:, 4:8, :], xt_r[0][:, 4:8, :])

        # ---- persistent tensors / constants ------------------------------
        wq_sb = sb.tile([128, 8, 128], BF16)
        wk_sb = sb.tile([128, 8, 128], BF16)
        wv_sb = sb.tile([128, 8, 128], BF16)
        nc.sync.dma_start(wq_sb[:], wq.ap().rearrange("(t p) m -> p t m", p=128))
        nc.sync.dma_start(wk_sb[:], wk.ap().rearrange("(t p) m -> p t m", p=128))
        nc.sync.dma_start(wv_sb[:], wv.ap().rearrange("(t p) m -> p t m", p=128))
        bq_sb = sb.tile([128, 1], F32)
        bk_sb = sb.tile([128, 1], F32)
        bv_sb = sb.tile([128, 1], F32)
        nc.sync.dma_start(bq_sb[:], bq.ap().rearrange("(p a) -> p a", a=1))
        nc.sync.dma_start(bk_sb[:], bk.ap().rearrange("(p a) -> p a", a=1))
        nc.sync.dma_start(bv_sb[:], bv.ap().rearrange("(p a) -> p a", a=1))
        bo_sb = sb.tile([1, D], BF16)
        wo_sb = sb.tile([128, 8, D], BF16)

        QT = sb.tile([128, S], BF16)      # rows 0-63 head0, 64-127 head1
        KT = sb.tile([128, S], BF16)
        # V' storage: per 128-seq block: [V_h0 (64) | 1 | V_h1 (64) | 1]
        Vp = sb.tile([128, NSB * 130], BF16)
        ones_f32 = sb.tile([128, 1], F32)
        nc.vector.memset(ones_f32[:], 1.0)
        ones_col = sb.tile([128, 1], BF16)
        nc.vector.tensor_copy(ones_col[:], ones_f32[:])

        tri_f32 = sb.tile([128, 128], F32)  # tri[pj, j] = 1 if j >= pj else 0
        nc.gpsimd.memset(tri_f32[:], 1.0)
        nc.gpsimd.affine_select(
            out=tri_f32[:], in_=tri_f32[:], compare_op=mybir.AluOpType.is_ge,
            fill=0.0, base=0, pattern=[[1, 128]], channel_multiplier=-1)
        tri = sb.tile([128, 128], BF16)
        nc.vector.tensor_copy(tri[:], tri_f32[:])
        ident = sb.tile([128, 128], F32)
        make_identity(nc, ident[:])
        ones_row = sb.tile([1, 128], F32)
        nc.vector.memset(ones_row[:], 1.0)
        ones_sb = sb.tile([1, 128], BF16)
        nc.vector.tensor_copy(ones_sb[:], ones_row[:])
        wrow_f32 = sb.tile([1, 512], F32)
        nc.vector.memset(wrow_f32[:], 0.001)
        warm_row = sb.tile([1, 512], BF16)
        nc.vector.tensor_copy(warm_row[:], wrow_f32[:])

        # ---- PE warm-up: keep the array busy while the first x^T chunk is
        # in flight (HAM un-throttles only under sustained matmul activity).
        # Short N=128 matmuls: enough sustained activity to trip the 3.4us
        # window without delaying the first real projection behind them.
        warm_ps = ps_big.tile([128, 1024], F32, tag="st", name="warm_ps")
        for i in range(24):
            nc.tensor.matmul(warm_ps[:, 0:128], ones_sb[0:1, :],
                             warm_row[0:1, 0:128], start=True, stop=True)

        # Output ownership is interleaved so the AllToAll can be split into
        # NG pipelined exchanges: group g spans q-cols [1024g, 1024(g+1));
        # within it rank r owns cols [1024g+128r, 1024g+128(r+1)). Payload
        # rows 0-63: normalized O^T head0, rows 64-127: head1 (bf16).
        NG = SC // 2
        GW = 2 * QC // N_CORES
        a2a_in = [dram.tile([N_CORES, 128, GW], BF16, name=f"a2ain{g}")
                  for g in range(NG)]
        a2a_out = [dram.tile([N_CORES, 128, GW], BF16, name=f"a2aout{g}")
                   for g in range(NG)]
        # DRAM bounce buffers for the per-chunk softmax-recip broadcast
        rc_d = [dram.tile([1024], F32, name=f"rcd{c}") for c in range(SC)]

        # tiny warm-up exchange: absorbs the communicator-init barrier and
        # first-collective overhead while the early QKV chunks compute
        warm_in = dram.tile([N_CORES, 32], F32)
        warm_out = dram.tile([N_CORES, 32], F32)
        nc.gpsimd.collective_compute(
            "AllToAll", mybir.AluOpType.bypass,
            replica_groups=[list(range(N_CORES))],
            ins=[warm_in.opt()], outs=[warm_out.opt()])

        def make_qkv_bursts(c, xt_sb=None):
            """Per-chunk QKV work as small PE bursts, interleaved between
            attention slots of the previous chunk."""
            if xt_sb is None:
                xt_sb = sbx.tile([128, 8, QC], BF16, tag="xt", name=f"xt{c}")
                nc.sync.dma_start(xt_sb[:], xt_r[c])
            cs = slice(c * QC, (c + 1) * QC)
            st8 = {}

            def proj_burst(w_sb, b_sb, dst):
                def run():
                    p_ps = ps_big.tile([128, 1024], F32, tag="st",
                                       name=f"qkv{c}_{dst.name}")
                    for t in range(8):
                        nc.tensor.matmul(p_ps[:, 0:512], w_sb[:, t, :],
                                         xt_sb[:, t, :],
                                         start=(t == 0), stop=(t == 7))
                    nc.vector.tensor_scalar_add(dst, p_ps[:, 0:512], b_sb[:])
                return run

            def q_burst():
                proj_burst(wq_sb, bq_sb, QT[:, cs])()
            def k_burst():
                proj_burst(wk_sb, bk_sb, KT[:, cs])()
            def v_burst():
                vt_sb = sbtmp.tile([128, QC], F32, tag="vt", name=f"vt{c}")
                st8["vt"] = vt_sb
                proj_burst(wv_sb, bv_sb, vt_sb[:])()

            def t_burst(sbk):
                def run():
                    blk = c * 4 + sbk
                    vt_sb = st8["vt"]
                    tp_ps = ps_big.tile([128, 128], F32, tag="st",
                                        name=f"tp{blk}")
                    nc.tensor.transpose(
                        tp_ps[:], vt_sb[:, sbk * 128:(sbk + 1) * 128], ident[:])
                    nc.vector.tensor_copy(Vp[:, blk * 130: blk * 130 + 64],
                                          tp_ps[:, 0:64])
                    nc.vector.tensor_copy(Vp[:, blk * 130 + 65: blk * 130 + 129],
                                          tp_ps[:, 64:128])
                    nc.vector.tensor_copy(Vp[:, blk * 130 + 64: blk * 130 + 65],
                                          ones_col[:])
                    nc.vector.tensor_copy(Vp[:, blk * 130 + 129: blk * 130 + 130],
                                          ones_col[:])
                return run

            return [q_burst, k_burst, v_burst,
                    t_burst(0), t_burst(1), t_burst(2), t_burst(3)]

        def emit_group_half(g, n2, of_sb):
            op_ps = ps_big.tile([128, 512], F32, tag="st",
                                name=f"op{g}_{n2}")
            for s in range(8):
                nc.tensor.matmul(
                    op_ps[:], of_sb[:, s, 0:128],
                    wo_sb[:, s, n2 * 512:(n2 + 1) * 512],
                    start=(s == 0), stop=False)
            nc.tensor.matmul(op_ps[:], ones_sb[0:1, :],
                             bo_sb[0:1, n2 * 512:(n2 + 1) * 512],
                             start=False, stop=True)
            o_sb = sbtmp.tile([128, 512], F32, tag="osb", name=f"o{g}_{n2}")
            nc.vector.tensor_copy(o_sb[:], op_ps[:])
            nc.sync.dma_start(
                out.ap()[g * 128:(g + 1) * 128, n2 * 512:(n2 + 1) * 512],
                o_sb[:])

        def emit_group_actions(g):
            """Output projection for group g as a list of deferred actions."""
            of_sb = sbtmp.tile([128, 8, GW], BF16, tag="of", name=f"of{g}")

            def d_load():
                # gpsimd queue: this DMA waits on the exchange's completion,
                # and on the sync queue it would head-of-line block the xt
                # prefetch and staging DMAs behind it
                nc.gpsimd.dma_start(
                    of_sb[:], a2a_out[g][:].rearrange("s p q -> p s q"))
            return [d_load,
                    lambda: emit_group_half(g, 0, of_sb),
                    lambda: emit_group_half(g, 1, of_sb)]

        # ------------------------------------------------------------------
        # software-pipelined attention
        # ------------------------------------------------------------------
        deferred = []
        pending = None          # (c, kb, pt, ot, is_last)

        def chunk_tail(c, ot):
            """Emitted immediately after the chunk's last AV matmuls: copy the
            accumulator out of PSUM (frees ot for the next chunk), then queue
            the normalize/stage/exchange steps."""
            on_sb = sbtmp.tile([65, 1024], F32, tag="on", name=f"on{c}")
            nc.vector.tensor_copy(on_sb[:], ot[:])

            bc_sb = sbnrm.tile([64, 1024], F32, tag="bc", name=f"bc{c}")
            on_nrm = sbnrm.tile([64, 1024], BF16, tag="nrm", name=f"nrm{c}")

            def d_recip():
                # ship the sums row to DRAM; DVE ops misbehave at partition
                # base 64, so all DVE work happens at base 0 post-broadcast
                nc.sync.dma_start(rc_d[c].rearrange("(a n) -> a n", a=1),
                                  on_sb[64:65, :])
            def d_bcast():
                nc.sync.dma_start(
                    bc_sb[:],
                    rc_d[c].rearrange("(a n) -> a n", a=1)
                    .to_broadcast((64, 1024)))
            def d_norm():
                nc.vector.reciprocal_approx_fast(bc_sb[:], bc_sb[:])
                nc.vector.tensor_mul(on_nrm[:], on_sb[0:64, :], bc_sb[:])
            def d_stage():
                g, jj = c // 2, c % 2
                npc = QC // GW          # owner pieces per chunk (4)
                nrm_r = on_nrm[:].rearrange("p (h q) -> p h q", h=2)
                for i in range(npc):
                    dst = npc * jj + i
                    nc.sync.dma_start(
                        a2a_in[g][dst].rearrange("(h p) q -> p h q", h=2),
                        nrm_r[:, :, i * GW:(i + 1) * GW])
                if jj == 1:
                    nc.gpsimd.collective_compute(
                        "AllToAll", mybir.AluOpType.bypass,
                        replica_groups=[list(range(N_CORES))],
                        ins=[a2a_in[g].opt()], outs=[a2a_out[g].opt()])
                if c == 1:
                    nc.sync.dma_start(
                        bo_sb[:], bo.ap().rearrange("(a n) -> a n", a=1))
                    nc.sync.dma_start(
                        wo_sb[:], wo.ap().rearrange("(t p) n -> p t n", p=128))

            deferred.extend([d_recip, d_bcast, d_norm, d_stage])
            # group g's exchange fires at the tail of chunk 2g+1; queue its
            # projection late enough that the exchange has surely completed
            # (the first exchange pays ~60us of communicator spin-up), and
            # place g2's into chunk 7, which has no QKV prefetch to fill it.
            emit_sched = {3: 0, 4: 1, 6: 2}
            if c in emit_sched:
                deferred.extend(emit_group_actions(emit_sched[c]))

        def do_av(av):
            c, kb, pt, ot, is_last = av
            nkb = 4 * (c + 1)
            ots = [ot[:, 0:512], ot[:, 512:1024]]
            t = kb - 4 * c
            off = 128 * t if t > 0 else 0  # fully-masked cols skipped
            for h in range(2):
                nc.tensor.matmul(
                    ots[h][:, off:512],
                    Vp[:, kb * 130 + h * 65: kb * 130 + (h + 1) * 65],
                    pt[:, h * 512 + off:(h + 1) * 512],
                    start=(kb == 0), stop=(kb == nkb - 1))
            if is_last:
                chunk_tail(c, ot)

        for b in make_qkv_bursts(0, xt0_sb):
            b()
        for c in range(SC):
            bursts = make_qkv_bursts(c + 1) if c + 1 < SC else []
            nb = len(bursts)
            done = 0
            cs = slice(c * QC, (c + 1) * QC)
            nkb = 4 * (c + 1)
            ot = ps_ot.tile([65, 1024], F32, tag="ot", name=f"ot{c}")
            for kb in range(nkb):
                # ---- scores for key-block kb, both heads ganged into one
                # tile (cols 0-511 h0, 512-1023 h1); the two matmuls target
                # different PE row-groups and run concurrently.
                st = ps_big.tile([128, 1024], F32, tag="st",
                                 name=f"st{c}_{kb}")
                for h in range(2):
                    hs = slice(h * 64, (h + 1) * 64)
                    nc.tensor.matmul(
                        st[:, h * 512:(h + 1) * 512],
                        KT[hs, kb * 128:(kb + 1) * 128],
                        QT[hs, cs], start=True, stop=True)
                # ---- exp on the Scalar engine (one FD-1024 ACTIVATE/slot)
                pt = sbpt.tile([128, 1024], BF16, tag="pt",
                               name=f"pt{c}_{kb}")
                nc.scalar.activation(pt[:], st[:], EXP, scale=0.125)
                # ---- dependency-free PE work goes in the queue BEFORE the
                # AV matmul below, so the PE streams while exp(s-1) finishes
                want = (kb + 1) * nb // nkb
                while done < want:
                    bursts[done]()
                    done += 1
                if deferred:
                    deferred.pop(0)()
                # ---- AV of the previous slot (one-deep ladder). If it closes
                # a chunk, the PSUM-accumulator copy is emitted right here so
                # it precedes this slot's mask on the in-order DVE queue.
                if pending is not None:
                    do_av(pending)
                # ---- causal mask on the diagonal block
                t = kb - 4 * c
                if t >= 0:
                    for h in range(2):
                        ms = slice(h * 512 + 128 * t, h * 512 + 128 * t + 128)
                        nc.vector.tensor_mul(pt[:, ms], pt[:, ms], tri[:])
                pending = (c, kb, pt, ot, kb == nkb - 1)

        do_av(pending)
        while deferred:
            deferred.pop(0)()
        for act in emit_group_actions(NG - 1):
            act()

    nc.compile()
    return nc


_NC_CACHE = {}


def _get_nc(S):
    if S not in _NC_CACHE:
        _NC_CACHE[S] = build(S)
    return _NC_CACHE[S]


def kernel(x, mask, Wq, bq, Wk, bk, Wv, bv, Wo, bo):
    import ml_dtypes
    x = np.asarray(x, np.float32)
    S = x.shape[1]
    xt = np.ascontiguousarray(x[0].T).astype(ml_dtypes.bfloat16)  # [D, S]
    Wq, Wk, Wv, Wo = (np.asarray(w, np.float32) for w in (Wq, Wk, Wv, Wo))
    bq, bk, bv, bo = (np.asarray(b, np.float32) for b in (bq, bk, bv, bo))
    # mask is structurally causal (jnp.tril in the reference); handled on-device.

    in_maps = []
    for r in range(N_CORES):
        sl = slice(128 * r, 128 * (r + 1))
        in_maps.append({
            "xt": xt,
            "wq": np.ascontiguousarray(Wq[:, sl]).astype(ml_dtypes.bfloat16),
            "wk": np.ascontiguousarray(Wk[:, sl]).astype(ml_dtypes.bfloat16),
            "wv": np.ascontiguousarray(Wv[:, sl]).astype(ml_dtypes.bfloat16),
            "wo": Wo.astype(ml_dtypes.bfloat16),
            "bq": np.ascontiguousarray(bq[sl]),
            "bk": np.ascontiguousarray(bk[sl]),
            "bv": np.ascontiguousarray(bv[sl]),
            "bo": bo.astype(ml_dtypes.bfloat16),
        })
    nc = _get_nc(S)
    global LAST_RESULT
    LAST_RESULT = run_bass_kernel_spmd(nc, in_maps, list(range(N_CORES)),
                                       trace=TRACE)
    res = LAST_RESULT.results
    # shard rows are (group, piece) interleaved: shard row g*GW+i of rank r
    # holds global row 2*QC*g + GW*r + i
    GW = 2 * QC // N_CORES
    NG = S // (2 * QC)
    stacked = np.stack([res[r]["out"].reshape(NG, GW, D)
                        for r in range(N_CORES)], axis=1)
    return stacked.reshape(S, D)[None].astype(np.float32)


TRACE = False          # test harness flips this to profile
LAST_RESULT = None
